# revision 18
# baseline (speedup 1.0000x reference)
"""Trainium2 Bass kernel for nn_ByteToLatentAttention.

Sharding: 8 cores = 2 (batch) x 4 (head-groups of 4 heads).  Each core
computes a partial output  attn_part @ wout_rows + merged_raw_rows @ wbyp_rows
for its batch; the host sums the 4 partials per batch and adds wout_b.

Precision: the bypass path (raw x @ wbyp), which dominates the output
magnitude, runs in fp32 (float32r matmuls).  The attention pipeline runs in
fp8e4m3 with DoubleRow matmuls (two 128-deep contraction planes per
instruction) and fp32 PSUM accumulation.  Attention contributes ~0.25% of
the output magnitude, so fp8 noise is diluted ~400x.

Layout tricks:
  * Q/K live in a plane-split layout [32*head + freq, plane] (plane = rope
    half), so rotate-half RoPE is pure elementwise math across the two
    plane tiles - no permutation matmuls.
  * Scores use DoubleRow with 32-partition lhsT tiles (4 heads stacked at
    partition offsets 0/32/64/96).
  * attn@V runs in [q, d] orientation with a constant column (=64)
    appended to V: the softmax denominator falls out of the same matmuls,
    and the divide is a per-partition tensor_scalar op.
  * Weight scaling: wq,wk,wv,wo are scaled by 8 host-side; the V-path
    scale cancels against the ones-column (=64), the QK scale is folded
    into the exp() scale, wo's into acT = attn/8.

Self-contained: hardcodes all shapes; uses only numpy + concourse.
"""

from contextlib import ExitStack

import numpy as np

import concourse.bass as bass
import concourse.tile as tile
from concourse import bacc
from concourse import mybir
from concourse.bass_utils import run_bass_kernel_spmd
from concourse.masks import make_identity

# ---- problem constants ----
B, S, D = 2, 4096, 512
BPL, H, DQK = 4, 16, 64
DLAT = 1024
LQ = S // BPL  # 1024
EPS = 1.1920929e-07
ROPE_BASE = 10000.0
NCORES = 8
NH = (H // 4) * DQK  # 256 features per core (4 heads)
P = 128
WS = 8.0  # host-side weight scale for fp8 quality

F32 = mybir.dt.float32
BF16 = mybir.dt.bfloat16
FP8 = mybir.dt.float8e4
I8 = mybir.dt.int8
MM_F32 = mybir.dt.float32r

AF = mybir.ActivationFunctionType
ALU = mybir.AluOpType
DR = mybir.MatmulPerfMode.DoubleRow

EXP_SCALE = 0.125 / (WS * WS)
EXP_BIAS = -1.0


def _kernel_body(ctx: ExitStack, tc, io):
    nc = tc.nc

    const = ctx.enter_context(tc.tile_pool(name="const", bufs=1))
    work = ctx.enter_context(tc.tile_pool(name="work", bufs=2))
    evp = ctx.enter_context(tc.tile_pool(name="evp", bufs=3))
    stage = ctx.enter_context(tc.tile_pool(name="stage", bufs=1))
    psS = ctx.enter_context(tc.tile_pool(name="psS", bufs=3, space="PSUM"))
    psAcc = ctx.enter_context(tc.tile_pool(name="psAcc", bufs=1, space="PSUM"))

    ident_h = const.tile([P, P], BF16)
    make_identity(nc, ident_h)
    ones8 = const.tile([P, 2, P], FP8)
    nc.vector.memset(ones8, 1.0)
    eps_sb = const.tile([P, 1], F32)
    nc.vector.memset(eps_sb, EPS)
    expb_sb = const.tile([P, 1], F32)
    nc.vector.memset(expb_sb, EXP_BIAS)

    bq_sb = const.tile([P, 2], F32)
    nc.sync.dma_start(out=bq_sb, in_=io["bq"])
    bk_sb = const.tile([P, 2], F32)
    nc.sync.dma_start(out=bk_sb, in_=io["bk"])
    bvrow = const.tile([P, NH], BF16)
    nc.sync.dma_start(out=bvrow, in_=io["bvrow"])

    # persistent big tensors
    normXT = const.tile([P, 4, S], FP8)  # [d_p, dc, s]  normalized x^T
    bypT = const.tile([P, 4, LQ], MM_F32)  # [d_p, dc, l]  raw bypass rows^T
    QT8 = const.tile([P, 2, LQ], FP8)  # [32h+f, plane, l]  roped Q (x8)
    KT8 = const.tile([P, 2, S], FP8)  # [32h+f, plane, s]  roped K (x8)
    Vn = const.tile([P, 32, 4 * 65], FP8)  # [s_p, sc, 65*h(+den col)] V (x8)
    acT = const.tile([P, 2, LQ], BF16)  # [d_p, plane, q]  attn transposed

    # denominator column (=WS, cancels the x8 V scale) interleaved at 65h+64
    nc.vector.memset(Vn.rearrange("p c (h x) -> p c h x", x=65)[:, :, :, 64], WS)

    # ---------- phase 0: PE warm-up (p-state ramp) ----------
    for wi in range(32):
        warm_ps = psS.tile([P, 1024], F32, tag="sc")
        nc.tensor.matmul(
            warm_ps[:, 0:128], lhsT=ident_h, rhs=ident_h, start=True, stop=True
        )

    # ---------- phase 1: RMS norm in transposed domain ----------
    xT = stage.tile([P, 4, S], FP8, tag="xT")
    nc.sync.dma_start(out=xT, in_=io["x_b"])
    nc.sync.dma_start(out=bypT, in_=io["x_byp"])

    for sc8 in range(8):
        ssl = slice(sc8 * 512, (sc8 + 1) * 512)
        sq = work.tile([P, 4, 512], FP8, tag="sq")
        for dc in range(4):
            eng = nc.gpsimd if dc % 2 == 0 else nc.vector
            eng.tensor_mul(out=sq[:, dc, :], in0=xT[:, dc, ssl], in1=xT[:, dc, ssl])
        pss = psS.tile([P, 1024], F32, tag="sc")
        for pr in range(2):
            nc.tensor.matmul(
                pss[:, 0:512],
                lhsT=ones8,
                rhs=sq[:, 2 * pr : 2 * pr + 2, :],
                start=(pr == 0),
                stop=(pr == 1),
                perf_mode=DR,
            )
        rmsb = work.tile([P, 512], F32, tag="rmsb")
        nc.scalar.activation(
            out=rmsb, in_=pss[:, 0:512], func=AF.Sqrt, bias=eps_sb, scale=1.0 / D
        )
        rinvf = work.tile([P, 512], F32, tag="rinvf")
        nc.vector.reciprocal_approx_fast(out=rinvf, in_=rmsb)
        for dc in range(4):
            eng = nc.gpsimd if dc % 2 == 0 else nc.vector
            eng.tensor_mul(out=normXT[:, dc, ssl], in0=xT[:, dc, ssl], in1=rinvf)

    # ---------- phase 2: K projection + rope ----------
    wk_sb = stage.tile([P, 2, 2, 2, P], FP8, tag="wk")
    nc.sync.dma_start(out=wk_sb, in_=io["wk"])
    csk = stage.tile([P, 2, S], BF16, tag="csk")
    nc.sync.dma_start(out=csk[:, 0, :], in_=io["cosk"])
    nc.sync.dma_start(out=csk[:, 1, :], in_=io["sink"])

    def rope_pair(p0, p1, dstT, ssl, cos, sin, bias):
        # evac psum with bias (scalar engine), bf16
        kb0 = work.tile([P, 512], BF16, tag="kb0")
        nc.scalar.add(out=kb0, in_=p0, add=bias[:, 0:1])
        kb1 = work.tile([P, 512], BF16, tag="kb1")
        nc.scalar.add(out=kb1, in_=p1, add=bias[:, 1:2])
        t1 = work.tile([P, 512], BF16, tag="t1")
        nc.vector.tensor_mul(out=t1, in0=kb0, in1=cos)
        t2 = work.tile([P, 512], BF16, tag="t2")
        nc.vector.tensor_mul(out=t2, in0=kb1, in1=sin)
        nc.vector.tensor_sub(out=dstT[:, 0, ssl], in0=t1, in1=t2)
        t3 = work.tile([P, 512], BF16, tag="t3")
        nc.gpsimd.tensor_mul(out=t3, in0=kb1, in1=cos)
        t4 = work.tile([P, 512], BF16, tag="t4")
        nc.gpsimd.tensor_mul(out=t4, in0=kb0, in1=sin)
        nc.gpsimd.tensor_add(out=dstT[:, 1, ssl], in0=t3, in1=t4)

    for sf in range(8):
        ssl = slice(sf * 512, (sf + 1) * 512)
        pks = []
        for mq in range(2):
            pk = psS.tile([P, 1024], F32, tag="sc")
            for pr in range(2):
                nc.tensor.matmul(
                    pk[:, 0:512],
                    lhsT=wk_sb[:, pr, :, mq, :],
                    rhs=normXT[:, 2 * pr : 2 * pr + 2, ssl],
                    start=(pr == 0),
                    stop=(pr == 1),
                    perf_mode=DR,
                )
            pks.append(pk)
        rope_pair(
            pks[0][:, 0:512], pks[1][:, 0:512], KT8, ssl,
            csk[:, 0, ssl], csk[:, 1, ssl], bk_sb,
        )

    # ---------- phase 3: Q projection + rope ----------
    wq_sb = stage.tile([P, 8, 2, 2, P], FP8, tag="wq")
    nc.sync.dma_start(out=wq_sb, in_=io["wq"])
    csq = stage.tile([P, 2, LQ], BF16, tag="csq")
    nc.sync.dma_start(out=csq[:, 0, :], in_=io["cosq"])
    nc.sync.dma_start(out=csq[:, 1, :], in_=io["sinq"])

    # normXT as [p, dc, l, j] with s = 4l + j
    normQ = normXT.rearrange("p c (l j) -> p c l j", j=4)

    for qf in range(2):
        qsl = slice(qf * 512, (qf + 1) * 512)
        pqs = []
        for mq in range(2):
            pq = psS.tile([P, 1024], F32, tag="sc")
            for pr in range(8):
                jj = pr // 2
                a = 2 * (pr % 2)
                rhs = normQ[:, a : a + 2, qsl, jj]
                nc.tensor.matmul(
                    pq[:, 0:512],
                    lhsT=wq_sb[:, pr, :, mq, :],
                    rhs=rhs,
                    start=(pr == 0),
                    stop=(pr == 7),
                    perf_mode=DR,
                )
            pqs.append(pq)
        rope_pair(
            pqs[0][:, 0:512], pqs[1][:, 0:512], QT8, qsl,
            csq[:, 0, qsl], csq[:, 1, qsl], bq_sb,
        )

    # ---------- phase 4: V projection ----------
    wv_sb = stage.tile([P, 2, 2, NH], FP8, tag="wv")
    nc.sync.dma_start(out=wv_sb, in_=io["wv"])
    VnH = Vn.rearrange("p c (h x) -> p c h x", x=65)
    for sc in range(S // P):
        pv = psS.tile([P, 1024], F32, tag="sc")
        for pr in range(2):
            nc.tensor.matmul(
                pv[:, 0:NH],
                lhsT=normXT[:, 2 * pr : 2 * pr + 2, sc * P : (sc + 1) * P],
                rhs=wv_sb[:, pr, :, :],
                start=(pr == 0),
                stop=(pr == 1),
                perf_mode=DR,
            )
        nc.scalar.copy(
            out=VnH[:, sc, :, 0:64],
            in_=pv[:, 0:NH].rearrange("p (h x) -> p h x", x=64),
        )

    # ---------- phase 5: attention (+ transposed attn output) ----------
    wo_sb = stage.tile([P, 2, DLAT], BF16, tag="wo")
    nc.sync.dma_start(out=wo_sb, in_=io["wo"])
    wb_sb = stage.tile([P, 4, DLAT], MM_F32, tag="wb")
    nc.sync.dma_start(out=wb_sb, in_=io["wb"])

    pe_backlog = []  # deferred PE+evac work to interleave into attention

    def attention_block(qc, hp):
        # q block [qc*512, (qc+1)*512), heads (2hp, 2hp+1)
        qsl = slice(qc * 512, (qc + 1) * 512)
        pac = psAcc.tile([P, 2, 512], F32, tag="pac")

        def scores(t):
            # one sc pair -> e2 tile [128k, 2sc, 2head*512q] in fp8
            e2 = evp.tile([P, 2, 1024], FP8, tag="e2")
            for i in range(2):
                sc = 2 * t + i
                ksl = slice(sc * P, (sc + 1) * P)
                ps = psS.tile([P, 1024], F32, tag="sc")
                for h in range(2):
                    hh = 2 * hp + h
                    pp = slice(32 * hh, 32 * hh + 32)
                    nc.tensor.matmul(
                        ps[:, h * 512 : (h + 1) * 512],
                        lhsT=KT8[pp, :, ksl],
                        rhs=QT8[pp, :, qsl],
                        start=True,
                        stop=True,
                        perf_mode=DR,
                        skip_group_check=True,
                        tile_position=(32 * hh, 0),
                    )
                # exp: e = exp(scores/(8*8*8) - 1), fp8 out
                nc.scalar.activation(
                    out=e2[:, i, :], in_=ps, func=AF.Exp,
                    scale=EXP_SCALE, bias=expb_sb,
                )
            return e2

        def attnv(t, e2):
            st, sp = (t == 0), (t == 15)
            for h in range(2):
                for qt in range(4):
                    nc.tensor.matmul(
                        pac[:, h, qt * 128 : qt * 128 + 65],
                        lhsT=e2[:, :, h * 512 + qt * 128 : h * 512 + (qt + 1) * 128],
                        rhs=VnH[:, 2 * t : 2 * t + 2, 2 * hp + h, :],
                        start=st,
                        stop=sp,
                        perf_mode=DR,
                        skip_group_check=True,
                    )

        e2_cur = scores(0)
        for t in range(16):
            e2_next = scores(t + 1) if t < 15 else None
            attnv(t, e2_cur)
            e2_cur = e2_next
            if pe_backlog:
                pe_backlog.pop(0)()
        return pac

    def divide_block(qc, hp, pac, attnN):
        # attnN[:, qt, 64*(2hp+h)+j] = pac_num * (1/den) + bv/8
        for h in range(2):
            hh = 2 * hp + h
            for qt in range(4):
                rc = work.tile([P, 1], F32, tag="rc")
                nc.vector.reciprocal_approx_fast(
                    out=rc, in_=pac[:, h, qt * 128 + 64 : qt * 128 + 65]
                )
                nc.vector.scalar_tensor_tensor(
                    out=attnN[:, qt, 64 * hh : 64 * hh + 64],
                    in0=pac[:, h, qt * 128 : qt * 128 + 64],
                    scalar=rc,
                    in1=bvrow[:, 64 * hh : 64 * hh + 64],
                    op0=ALU.mult,
                    op1=ALU.add,
                )

    def transpose_evac(qc, attnN):
        # attnN [128q, qt, 256d] -> acT [128d, plane, q] via PE transpose
        for qt in range(4):
            for dh in range(2):
                def go(qt=qt, dh=dh):
                    psT = psS.tile([P, 1024], F32, tag="sc")
                    nc.tensor.matmul(
                        psT[:, 0:128],
                        lhsT=attnN[:, qt, dh * 128 : (dh + 1) * 128],
                        rhs=ident_h,
                        start=True,
                        stop=True,
                    )
                    qoff = qc * 512 + qt * 128
                    nc.vector.tensor_copy(
                        out=acT[:, dh, qoff : qoff + 128], in_=psT[:, 0:128]
                    )
                pe_backlog.append(go)

    def outproj(qc):
        for qt in range(4):
            qoff = qc * 512 + qt * 128

            def go(qoff=qoff):
                osb = evp.tile([P, DLAT], F32, tag="osb")
                for oc in range(2):
                    osl = slice(oc * 512, (oc + 1) * 512)
                    po = psS.tile([P, 1024], F32, tag="sc")
                    for pl in range(2):
                        nc.tensor.matmul(
                            po[:, 0:512],
                            lhsT=acT[:, pl, qoff : qoff + 128],
                            rhs=wo_sb[:, pl, osl],
                            start=(pl == 0),
                            stop=False,
                        )
                    for dc in range(4):
                        nc.tensor.matmul(
                            po[:, 0:512],
                            lhsT=bypT[:, dc, qoff : qoff + 128],
                            rhs=wb_sb[:, dc, osl],
                            start=False,
                            stop=(dc == 3),
                        )
                    nc.vector.tensor_copy(out=osb[:, osl], in_=po[:, 0:512])
                nc.sync.dma_start(
                    out=io["out_partial"][qoff : qoff + 128, :], in_=osb
                )

            pe_backlog.append(go)

    attnN_tiles = {}
    for qc in range(2):
        for hp in range(2):
            pac = attention_block(qc, hp)
            if hp == 0:
                attnN = work.tile([P, 4, NH], BF16, tag=f"attnN{qc}", name=f"attnN{qc}")
                attnN_tiles[qc] = attnN
            divide_block(qc, hp, pac, attnN_tiles[qc])
        transpose_evac(qc, attnN_tiles[qc])
        outproj(qc)
    while pe_backlog:
        pe_backlog.pop(0)()


def build_program():
    nc = bacc.Bacc("TRN2", target_bir_lowering=False, debug=False)
    io = {}

    def inp(name, shape, dtype=F32):
        io[name] = nc.dram_tensor(name, list(shape), dtype, kind="ExternalInput").ap()

    inp("x_b", [P, 4, S], FP8)
    inp("x_byp", [P, 4, LQ], MM_F32)
    inp("wq", [P, 8, 2, 2, P], FP8)
    inp("wk", [P, 2, 2, 2, P], FP8)
    inp("wv", [P, 2, 2, NH], FP8)
    inp("bq", [P, 2])
    inp("bk", [P, 2])
    inp("bvrow", [P, NH], BF16)
    inp("wo", [P, 2, DLAT], BF16)
    inp("wb", [P, 4, DLAT], MM_F32)
    inp("cosq", [P, LQ], BF16)
    inp("sinq", [P, LQ], BF16)
    inp("cosk", [P, S], BF16)
    inp("sink", [P, S], BF16)
    io["out_partial"] = nc.dram_tensor(
        "out_partial", [LQ, DLAT], F32, kind="ExternalOutput"
    ).ap()

    with tile.TileContext(nc) as tc:
        with ExitStack() as ctx:
            _kernel_body(ctx, tc, io)
    nc.compile()
    return nc


def _rope_tables_plane(pos):
    # plane layout: row p -> freq index p % 32 (shared by the 4 heads)
    half = DQK // 2  # 32
    invfreq = ROPE_BASE ** (-np.arange(half, dtype=np.float64) / half)
    ang = pos[None, :].astype(np.float64) * invfreq[:, None]  # [32, L]
    cos = np.tile(np.cos(ang), (4, 1))  # [128, L]
    sin = np.tile(np.sin(ang), (4, 1))
    return cos, sin


def _tf32(a):
    u = np.ascontiguousarray(np.asarray(a, dtype=np.float32)).view(np.uint32)
    lsb = (u >> np.uint32(13)) & np.uint32(1)
    u = (u + np.uint32(0x0FFF) + lsb) & np.uint32(0xFFFFE000)
    return u.view(np.float32)


def _bf16(a):
    import ml_dtypes

    return np.ascontiguousarray(np.asarray(a).astype(ml_dtypes.bfloat16))


def _fp8(a):
    import ml_dtypes

    return np.ascontiguousarray(np.asarray(a).astype(ml_dtypes.float8_e4m3))


def make_in_map(core, inputs):
    b, hg = core // 4, core % 4
    x = np.asarray(inputs["x"], dtype=np.float32)
    nw = np.asarray(inputs["norm_w"], dtype=np.float32)
    wq_w = np.asarray(inputs["wq_w"], dtype=np.float32)
    wq_b = np.asarray(inputs["wq_b"], dtype=np.float32)
    wkv_w = np.asarray(inputs["wkv_w"], dtype=np.float32)
    wkv_b = np.asarray(inputs["wkv_b"], dtype=np.float32)
    wout_w = np.asarray(inputs["wout_w"], dtype=np.float32)
    wbyp_w = np.asarray(inputs["wbyp_w"], dtype=np.float32)

    nsl = slice(hg * NH, (hg + 1) * NH)
    vsl = slice(H * DQK + hg * NH, H * DQK + (hg + 1) * NH)
    wq_c = wq_w * np.tile(nw, BPL)[:, None]
    wkv_c = wkv_w * nw[:, None]

    # plane-permuted column order for Q/K: col (pl, p) -> n = 64*(p//32)+32*pl+p%32
    p_idx = np.arange(P)
    perm = [64 * (p_idx // 32) + 32 * pl + (p_idx % 32) for pl in range(2)]

    # wq chunks: c = 4*j + dc; pr = c//2, plc = c%2 -> rows 512*j + dc*128 + dp
    wq8 = np.zeros((P, 8, 2, 2, P), dtype=np.float32)
    wk8 = np.zeros((P, 2, 2, 2, P), dtype=np.float32)
    wqs = wq_c[:, nsl] * WS
    wks = wkv_c[:, nsl] * WS
    for pr in range(8):
        for plc in range(2):
            jj, dc = pr // 2, 2 * (pr % 2) + plc
            rows = slice(512 * jj + dc * 128, 512 * jj + dc * 128 + 128)
            for mq in range(2):
                wq8[:, pr, plc, mq, :] = wqs[rows, :][:, perm[mq]]
    for pr in range(2):
        for plc in range(2):
            dc = 2 * pr + plc
            rows = slice(dc * 128, dc * 128 + 128)
            for mq in range(2):
                wk8[:, pr, plc, mq, :] = wks[rows, :][:, perm[mq]]
    wv8 = np.zeros((P, 2, 2, NH), dtype=np.float32)
    wvs = wkv_c[:, vsl] * WS
    for pr in range(2):
        for plc in range(2):
            dc = 2 * pr + plc
            wv8[:, pr, plc, :] = wvs[dc * 128 : dc * 128 + 128, :]

    bq8 = np.stack([wq_b[nsl][perm[0]], wq_b[nsl][perm[1]]], axis=1) * WS
    bk8 = np.stack([wkv_b[nsl][perm[0]], wkv_b[nsl][perm[1]]], axis=1) * WS
    bvr = np.tile(wkv_b[vsl][None, :], (P, 1))

    wo2 = wout_w[nsl, :].reshape(2, P, DLAT).transpose(1, 0, 2)

    cosq, sinq = _rope_tables_plane(np.arange(LQ) * float(BPL))
    cosk, sink = _rope_tables_plane(np.arange(S).astype(np.float64))

    return {
        "x_b": _fp8(x[b].T.reshape(4, P, S).transpose(1, 0, 2)),
        "x_byp": _tf32(
            np.ascontiguousarray(x[b, hg::BPL, :].T.reshape(4, P, LQ).transpose(1, 0, 2))
        ),
        "wq": _fp8(wq8),
        "wk": _fp8(wk8),
        "wv": _fp8(wv8),
        "bq": np.ascontiguousarray(bq8, dtype=np.float32),
        "bk": np.ascontiguousarray(bk8, dtype=np.float32),
        "bvrow": _bf16(bvr),
        "wo": _bf16(wo2),
        "wb": _tf32(
            np.ascontiguousarray(
                wbyp_w[hg * D : (hg + 1) * D, :].reshape(4, P, DLAT).transpose(1, 0, 2)
            )
        ),
        "cosq": _bf16(cosq),
        "sinq": _bf16(sinq),
        "cosk": _bf16(cosk),
        "sink": _bf16(sink),
    }


_nc_cache = None


def _get_program():
    global _nc_cache
    if _nc_cache is None:
        _nc_cache = build_program()
    return _nc_cache


def run_device(inputs, trace=False):
    nc = _get_program()
    in_maps = [make_in_map(c, inputs) for c in range(NCORES)]
    res = run_bass_kernel_spmd(nc, in_maps, core_ids=list(range(NCORES)), trace=trace)
    return res


def assemble(parts, inputs):
    wout_b = np.asarray(inputs["wout_b"], dtype=np.float32)
    out = np.zeros((B, LQ, DLAT), dtype=np.float64)
    for c in range(NCORES):
        out[c // 4] += np.asarray(parts[c], dtype=np.float64)
    out += wout_b[None, None, :].astype(np.float64)
    return out.astype(np.float32)


def kernel(**inputs):
    res = run_device(inputs)
    parts = [r["out_partial"] for r in res.results]
    return assemble(parts, inputs)
